# revision 59
# baseline (speedup 1.0000x reference)
"""Self-contained Trainium2 Bass kernel for a 12-head attention layer.

Problem: x[4,2048,768] -> attention(QKV projections, softmax, context),
NUM_HEADS=12, SIZE_PER_HEAD=64, additive mask from mask[4,2048].

Sharding over 8 NeuronCores: core c handles batch b=c//2 and head-group
hg=c%2 (6 heads, 384 feature columns).  Everything is local per core:
no collectives.

Design (ACT-bound): the per-core exp work (6 heads x 2048^2 = 25.2M
elements at 1 elem/lane/cycle @1.2GHz + ~350c/inst) floors the Scalar
engine at ~200us busy with N=1024 ACTs, so everything else is arranged
to hide under that roof:

  one flattened software pipeline over all 192 (pair p, f-chunk g of
  512, t-tile ti) steps; per step:
    scores  S^T[t,f]: head A -> psS[:,0:512], head B -> psS[:,512:1024]
            (one [128,1024] psum tile = two banks, double-buffered;
            the two K=64 matmuls row-tile-overlap partially)
    exp     ONE ACT N=1024 over both heads' chunks:
            Exp(psS + adder[ti]) -> bf16 [128,1024]  (mask = ACT bias,
            zero for the all-ones mask)
    ctx     emitted with a LAG=16-step delay so the PE FIFO never
            blocks on the ACT: per head one [65,512] psum accumulated
            over ti; V carries a leading ones-column so psum row 0 is
            the softmax denominator (no separate denominator matmuls);
            segment s's ctx drains exactly during segment s+1's scores,
            keeping the ctx psum tiles single-buffered.  Each segment's
            ti=0 ctx is deferred one extra step (its bank waits on the
            previous epilogue's drain copy); the LAST segment runs at
            lag 3 in the by-then-idle projection psum banks, shrinking
            the post-exp drain from 16 steps to 3
  the Q/K projections for later pairs drip in as one-instruction
  background thunks in the PE slack under the ACT cadence; the first
  four V-projection chains run during the input-DMA window in the
  then-idle scores psum banks (pipelining with the x chunk arrivals),
  the rest as two blocks behind the first segment's scores, hidden by
  the ACT's exp backlog.  Input DMA is ~14us (4.9MB at
  ~358GB/s) and is covered by garbage warm-up matmuls that release the
  HAM clock throttle; weights land before x so the first projection
  chains pipeline with the x-chunk arrivals.

  normalization fully off the PE: DVE psum drain, sbuf-sbuf DMA gather
  of the denominator rows into [128,n] so the reciprocal uses all DVE
  lanes, DMA scatter back to a row, gpsimd partition_broadcast, DVE
  multiply, DMA out.  The last segment uses 256-wide chains so the
  kernel tail pipelines across DVE/DMA/GpSimd.

Output per core: ctx^T [384,2048] f32; host transposes/concats.
Measured: ~257us HW exec (session baseline: 305-361us), rel err
3.468e-3.
"""

import numpy as np
import ml_dtypes

B, S, D = 4, 2048, 768
H, DH = 12, 64
HL = 6          # heads per core
DL = HL * DH    # 384 feature columns per core
NCORES = 8
P = 128
KO = 6          # full k-subtiles of the 768 contraction
NT = S // P     # 16 T-tiles
NG = 4          # f-chunks of 512 per head

_CACHE = {}


def _build(with_bias=False, ncores=NCORES, use_adder=True):
    import concourse.mybir as mybir
    import concourse.tile as tile
    from concourse import bacc

    dt = mybir.dt
    Exp = mybir.ActivationFunctionType.Exp
    Alu = mybir.AluOpType

    nc = bacc.Bacc("TRN2", target_bir_lowering=False, debug=False,
                   num_devices=ncores)

    DE = D + 1 if with_bias else D
    WVC = HL * (DH + 1) if with_bias else DL   # 390 vs 384
    xT = nc.dram_tensor("xT", [DE, S], dt.bfloat16, kind="ExternalInput")
    wq = nc.dram_tensor("wq", [DE, DL], dt.bfloat16, kind="ExternalInput")
    wk = nc.dram_tensor("wk", [DE, DL], dt.bfloat16, kind="ExternalInput")
    wv = nc.dram_tensor("wv", [DE, WVC], dt.bfloat16, kind="ExternalInput")
    adder = nc.dram_tensor("adder", [P, NT], dt.float32, kind="ExternalInput")
    out = nc.dram_tensor("out", [DL, S], dt.float32, kind="ExternalOutput")

    KE = KO + 1 if with_bias else KO

    with tile.TileContext(nc) as tc:
        with (
            tc.tile_pool(name="persist", bufs=1) as sb,
            tc.tile_pool(name="work", bufs=3) as work,
            tc.tile_pool(name="fin", bufs=2) as fin,
            tc.tile_pool(name="ps_s", bufs=2, space="PSUM") as ps_s,
            tc.tile_pool(name="ps_c", bufs=1, space="PSUM") as ps_c,
        ):
            # ---- input DMA ----
            xTs = sb.tile([P, KE, S], dt.bfloat16, tag="xTs")
            wqs = sb.tile([P, KE, DL], dt.bfloat16, tag="wqs")
            wks = sb.tile([P, KE, DL], dt.bfloat16, tag="wks")
            wvs = sb.tile([P, KE, WVC], dt.bfloat16, tag="wvs")
            adder_sb = sb.tile([P, NT], dt.float32, tag="adder")

            def dma_w(w_dram, w_sb, cols):
                # host already stores rows as [p, ko, m] (partition-major),
                # so this is a contiguous transfer, not a gather
                nc.sync.dma_start(
                    w_sb[:, 0:KO, 0:cols],
                    w_dram.ap()[0:D, :].rearrange("(p ko) m -> p ko m", p=P))
                if with_bias:
                    nc.sync.dma_start(w_sb[0:1, KO, 0:cols],
                                      w_dram.ap()[D:D + 1, :])

            # weights first: the first projection chains then pipeline
            # with the xT chunk arrivals instead of waiting for all of x
            dma_w(wq, wqs, DL)
            dma_w(wk, wks, DL)
            dma_w(wv, wvs, WVC)
            nc.sync.dma_start(adder_sb[:], adder.ap())
            for ko in range(KO):
                nc.sync.dma_start(
                    xTs[:, ko, :], xT.ap()[ko * P:(ko + 1) * P, :])
            if with_bias:
                nc.sync.dma_start(xTs[0:1, KO, :], xT.ap()[D:D + 1, :])

            # persistent projection outputs
            qt = sb.tile([P, 3, S], dt.bfloat16, tag="qt")   # Q^T/8 (+bias)
            kt = sb.tile([P, 3, S], dt.bfloat16, tag="kt")   # K^T (+bias)
            # V' token-major, 65-col head blocks (65th col = ones -> denom)
            # ones column FIRST in each head block: the ctx psum row 0 is
            # then the softmax denominator at partition 0, so the epilogue
            # reciprocal/broadcast needs no partition-moving DMAs
            vp = sb.tile([P, NT, HL, DH + 1], dt.bfloat16, tag="vp")
            if not with_bias:
                nc.gpsimd.memset(vp[:, :, :, 0:1], 1.0)

            # ---- projections ----
            def proj_qk(w_sb, dst, m, ns=(0, 1, 2, 3), tag="proj"):
                for n in ns:
                    pt = ps_c.tile([P, 512], dt.float32, tag=tag,
                                   name="pt", bufs=2 if tag == "proj" else 1)
                    for k in range(KE):
                        lhsT = (w_sb[:, k, m * P:(m + 1) * P] if k < KO
                                else w_sb[0:1, k, m * P:(m + 1) * P])
                        rhs = (xTs[:, k, n * 512:(n + 1) * 512] if k < KO
                               else xTs[0:1, k, n * 512:(n + 1) * 512])
                        nc.tensor.matmul(pt[:], lhsT, rhs,
                                         start=(k == 0), stop=(k == KE - 1))
                    nc.vector.tensor_copy(dst[:, m, n * 512:(n + 1) * 512],
                                          pt[:])

            def proj_v_into(mt, pt, off):
                for k in range(KE):
                    lhsT = (xTs[:, k, mt * P:(mt + 1) * P] if k < KO
                            else xTs[0:1, k, mt * P:(mt + 1) * P])
                    rhs = wvs[:, k, 0:WVC] if k < KO else wvs[0:1, k, 0:WVC]
                    nc.tensor.matmul(pt[:, off:off + WVC], lhsT, rhs,
                                     start=(k == 0), stop=(k == KE - 1))
                if with_bias:
                    nc.vector.tensor_copy(
                        vp[:, mt, :, :],
                        pt[:, off:off + WVC].rearrange("p (h c) -> p h c",
                                                       h=HL))
                else:
                    nc.vector.tensor_copy(
                        vp[:, mt, :, 1:DH + 1],
                        pt[:, off:off + DL].rearrange("p (h c) -> p h c",
                                                      h=HL))

            def proj_v(mt):
                pt = ps_c.tile([P, 512], dt.float32, tag="proj", name="pt",
                               bufs=2)
                proj_v_into(mt, pt, 0)

            def epilogue(p, g, ctx_ps, fine=False):
                # ctx_ps: [hip] -> psum tile; row 0 = denominator,
                # rows 1..64 = ctx.  Drain psum fast (DVE), then gather the
                # denom rows to [128,n] via sbuf-sbuf DMA so the reciprocal
                # uses all DVE lanes, scatter back, gpsimd-broadcast,
                # multiply, DMA out.  fine=True splits into 256-wide chains
                # so the kernel tail pipelines.
                csts = []
                for hip in range(2):
                    cst = fin.tile([DH + 1, 512], dt.float32,
                                   tag=f"cst{hip}", name="cst", bufs=2)
                    nc.vector.tensor_copy(cst[:], ctx_ps[hip][0:DH + 1, :])
                    csts.append(cst)
                nch = 2 if fine else 1
                cw = 512 // nch
                for ch in range(nch):
                    dcol = fin.tile([P, 8 // nch], dt.float32,
                                    tag=f"dcol{nch}", name="dcol",
                                    bufs=2 * nch)
                    for hip in range(2):
                        nc.sync.dma_start(
                            dcol[:, hip * 4 // nch:(hip + 1) * 4 // nch],
                            csts[hip][0:1, ch * cw:(ch + 1) * cw])
                    rc = fin.tile([P, 8 // nch], dt.float32,
                                  tag=f"rc{nch}", name="rc", bufs=2 * nch)
                    nc.vector.reciprocal(rc[:], dcol[:])
                    for hip in range(2):
                        rrow = fin.tile([1, cw], dt.float32,
                                        tag=f"rrow{nch}", name="rrow",
                                        bufs=2 * nch)
                        nc.sync.dma_start(
                            rrow[:],
                            rc[:, hip * 4 // nch:(hip + 1) * 4 // nch])
                        bb = fin.tile([DH + 1, cw], dt.float32,
                                      tag=f"bb{nch}", name="bb",
                                      bufs=2 * nch)
                        nc.gpsimd.partition_broadcast(bb[:], rrow[:])
                        otc = fin.tile([DH + 1, cw], dt.float32,
                                       tag=f"otc{nch}", bufs=3 * nch,
                                       name="otc")
                        nc.vector.tensor_tensor(
                            otc[:, :],
                            csts[hip][:, ch * cw:(ch + 1) * cw],
                            bb[:, :], Alu.mult)
                        nc.sync.dma_start(
                            out.ap()[p * P + hip * DH:
                                     p * P + (hip + 1) * DH,
                                     g * 512 + ch * cw:
                                     g * 512 + (ch + 1) * cw],
                            otc[1:DH + 1, :])

            def attn_all(bg, pre_step):
                # One flattened software pipeline over all 192
                # (pair, g, ti) steps: scores+exp for step j, ctx for step
                # j-16.  The segment-sized lag means ctx deps are a full
                # exp-backlog old (the PE FIFO never waits on the ACT),
                # segment s's ctx drains exactly during segment s+1's
                # scores (ctx psum tiles stay single-buffered), and every
                # segment/pair boundary bubble disappears.  pre_step maps
                # step -> emission block run before that step (V-proj
                # blocks that hide under the exp backlog); bg interleaves
                # one background-projection instruction per step.
                LAG = 16
                LATE = 176   # last segment: lag 3, ctx lives in the proj
                             # banks (free after the last projection), so
                             # the post-exp ctx drain shrinks 16 -> 3 steps
                ctx_tiles = {}
                exp_tiles = {}

                def seg_of(step):
                    p, r = divmod(step, 64)
                    return p, r // 16, r % 16

                def emit_at(j):
                    if j >= LATE:
                        return j + 3
                    # defer ti==0 one step: its psum tile waits on the
                    # previous segment's epilogue drain copy (bank WAR);
                    # emitting it a step later keeps the PE FIFO clear
                    return j + LAG + (1 if j % 16 == 0 else 0)

                for step in range(192 + LAG):
                    if step in pre_step:
                        pre_step[step]()
                    if step < 192:
                        p, g, ti = seg_of(step)
                        psS = ps_s.tile([P, 1024], dt.float32, tag="s",
                                        name="psS")
                        nc.tensor.matmul(
                            psS[:, 0:512],
                            kt[0:DH, p, ti * P:(ti + 1) * P],
                            qt[0:DH, p, g * 512:(g + 1) * 512],
                            start=True, stop=True)
                        nc.tensor.matmul(
                            psS[:, 512:1024],
                            kt[DH:P, p, ti * P:(ti + 1) * P],
                            qt[DH:P, p, g * 512:(g + 1) * 512],
                            start=True, stop=True)
                        et = work.tile([P, 1024], dt.bfloat16, tag="exp",
                                       name="et", bufs=LAG + 2)
                        nc.scalar.activation(
                            et[:], psS[:], Exp,
                            bias=(adder_sb[:, ti:ti + 1] if use_adder
                                  else 0.0),
                            scale=1.0)
                        exp_tiles[step] = et
                    for j in range(max(0, step - LAG - 1), step + 1):
                        if emit_at(j) != step or j >= 192:
                            continue
                        p, g, ti = seg_of(j)
                        seg = 4 * p + g
                        if ti == 0:
                            if j >= LATE:
                                ctx_tiles[seg] = [
                                    ps_c.tile([P, 512], dt.float32,
                                              tag="proj", name="ctx_ps",
                                              bufs=2)
                                    for hip in range(2)
                                ]
                            else:
                                ctx_tiles[seg] = [
                                    ps_c.tile([P, 512], dt.float32,
                                              tag=f"c{hip}",
                                              name="ctx_ps")
                                    for hip in range(2)
                                ]
                        et = exp_tiles.pop(j)
                        for hip in range(2):
                            nc.tensor.matmul(
                                ctx_tiles[seg][hip][0:DH + 1, :],
                                vp[:, ti, 2 * p + hip, :],
                                et[:, hip * 512:(hip + 1) * 512],
                                start=(ti == 0), stop=(ti == NT - 1))
                        if ti == NT - 1:
                            epilogue(p, g, ctx_tiles.pop(seg),
                                     fine=(seg == 11))
                    pops = 2 if step < 40 else 1
                    for _ in range(pops):
                        if bg:
                            bg.pop(0)()

            # PE warm-up: garbage matmuls with no input deps run during the
            # initial DMA wait, releasing the HAM clock throttle.
            warm = sb.tile([P, 512], dt.bfloat16, tag="warm")
            nc.gpsimd.memset(warm[:], 0.0)
            wexp = sb.tile([P, 1], dt.bfloat16, tag="wexp")
            nc.scalar.activation(wexp[:], warm[:, 0:1], Exp)
            wpt = ps_s.tile([P, 1024], dt.float32, tag="s", name="wpt")
            for wi in range(30):
                nc.tensor.matmul(wpt[:, 0:512], warm[:, 0:P], warm[:],
                                 start=(wi == 0), stop=(wi == 29))

            def proj_thunks_qk(w_sb, dst, m, n):
                # one-instruction-per-thunk version of proj_qk(m, (n,))
                state = {}

                def mk(k):
                    def t():
                        if k == 0:
                            state["pt"] = ps_c.tile([P, 512], dt.float32,
                                                    tag="proj", name="pt",
                                                    bufs=2)
                        lhsT = (w_sb[:, k, m * P:(m + 1) * P] if k < KO
                                else w_sb[0:1, k, m * P:(m + 1) * P])
                        rhs = (xTs[:, k, n * 512:(n + 1) * 512] if k < KO
                               else xTs[0:1, k, n * 512:(n + 1) * 512])
                        nc.tensor.matmul(state["pt"][:], lhsT, rhs,
                                         start=(k == 0), stop=(k == KE - 1))
                    return t

                def cp():
                    nc.vector.tensor_copy(
                        dst[:, m, n * 512:(n + 1) * 512], state["pt"][:])

                return [mk(k) for k in range(KE)] + [cp]

            # prefix: only what scores(0..3) need up front; remaining k-m0
            # chunks drip in just ahead of their ti via the bg queue (2
            # pops/step early on); the V projections run AFTER g0's first
            # scores, hidden under the ACT's exp backlog.
            proj_qk(wqs, qt, 0, ns=(0,))
            proj_qk(wks, kt, 0, ns=(0,), tag="c0")
            proj_qk(wks, kt, 0, ns=(1,), tag="c1")
            proj_qk(wks, kt, 0, ns=(2,))
            for vb in range(2):
                wv_ps = ps_s.tile([P, 1024], dt.float32, tag="s",
                                  name="wvps")
                for half in range(2):
                    proj_v_into(2 * vb + half, wv_ps, half * 512)
            proj_qk(wks, kt, 0, ns=(3,))

            def v_block(lo, hi):
                def f():
                    for mt in range(lo, hi):
                        proj_v(mt)
                return f

            def v_one(mt, tag):
                # single V chain in a briefly-idle ctx bank (free between
                # the head k-projection chains and ctx(0) at step ~17)
                def f():
                    pt = ps_c.tile([P, 512], dt.float32, tag=tag,
                                   name="pt", bufs=1)
                    proj_v_into(mt, pt, 0)
                return f

            bg = []
            for n in (1, 2, 3):
                bg += proj_thunks_qk(wqs, qt, 0, n)
            for n in range(4):
                bg += proj_thunks_qk(wks, kt, 1, n)
            for n in range(4):
                bg += proj_thunks_qk(wqs, qt, 1, n)
            for n in range(4):
                bg += proj_thunks_qk(wks, kt, 2, n)
            for n in range(4):
                bg += proj_thunks_qk(wqs, qt, 2, n)
            bg += [lambda: None] * (300 - len(bg))
            attn_all(bg, {16: v_block(4, 10), 24: v_block(10, NT)})

    nc.compile()
    return nc


def _prep_core_inputs(c, x, Wq, bq, Wk, bk, Wv, bv, mask, with_bias):
    bf16 = ml_dtypes.bfloat16
    b, hg = c // 2, c % 2
    cols = slice(hg * DL, (hg + 1) * DL)
    DE = D + 1 if with_bias else D

    xT_aug = np.empty((DE, S), dtype=bf16)
    xT_aug[:D] = x[b].T.astype(bf16)
    if with_bias:
        xT_aug[D] = np.float32(1.0)

    def pko(w):
        # store weight rows partition-major ([p, ko] instead of [ko, p])
        # so the on-chip DMA into [P, KO, cols] is contiguous
        c = w.shape[1]
        return w.reshape(KO, P, c).transpose(1, 0, 2).reshape(D, c)

    wq_aug = np.empty((DE, DL), dtype=bf16)
    wq_aug[:D] = pko((Wq[:, cols] / 8.0).astype(bf16))
    wk_aug = np.empty((DE, DL), dtype=bf16)
    wk_aug[:D] = pko(Wk[:, cols].astype(bf16))
    if with_bias:
        wq_aug[D] = (bq[cols] / 8.0).astype(bf16)
        wk_aug[D] = bk[cols].astype(bf16)
        wv_aug = np.zeros((DE, HL * (DH + 1)), dtype=bf16)
        wv_loc = Wv[:, cols].astype(np.float32)
        bv_loc = bv[cols].astype(np.float32)
        for j in range(HL):
            wv_aug[:D, j * (DH + 1) + 1:(j + 1) * (DH + 1)] = \
                wv_loc[:, j * DH:(j + 1) * DH].astype(bf16)
            wv_aug[D, j * (DH + 1) + 1:(j + 1) * (DH + 1)] = \
                bv_loc[j * DH:(j + 1) * DH].astype(bf16)
            wv_aug[D, j * (DH + 1)] = np.float32(1.0)
        wv_aug[:D] = pko(wv_aug[:D].copy())
    else:
        wv_aug = np.empty((DE, DL), dtype=bf16)
        wv_aug[:D] = pko(Wv[:, cols].astype(bf16))

    add = ((mask[b].astype(np.float32) - 1.0) * 10000.0)
    adder_t = add.reshape(NT, P).T.copy()   # [128,16]: [p, ti]

    return {"xT": xT_aug, "wq": wq_aug, "wk": wk_aug, "wv": wv_aug,
            "adder": np.ascontiguousarray(adder_t, dtype=np.float32)}


def kernel(x, Wq, bq, Wk, bk, Wv, bv, mask, _trace=False):
    from concourse.bass_utils import run_bass_kernel_spmd

    x = np.asarray(x, dtype=np.float32)
    Wq = np.asarray(Wq, dtype=np.float32)
    bq = np.asarray(bq, dtype=np.float32)
    Wk = np.asarray(Wk, dtype=np.float32)
    bk = np.asarray(bk, dtype=np.float32)
    Wv = np.asarray(Wv, dtype=np.float32)
    bv = np.asarray(bv, dtype=np.float32)
    mask = np.asarray(mask)

    with_bias = bool(bq.any() or bk.any() or bv.any())
    use_adder = not bool(np.asarray(mask).all())
    key = ("nc", with_bias, use_adder)
    if key not in _CACHE:
        _CACHE[key] = _build(with_bias=with_bias, use_adder=use_adder)
    nc = _CACHE[key]

    in_maps = [_prep_core_inputs(c, x, Wq, bq, Wk, bk, Wv, bv, mask,
                                 with_bias)
               for c in range(NCORES)]
    res = run_bass_kernel_spmd(nc, in_maps, core_ids=list(range(NCORES)),
                               trace=_trace)
    if _trace:
        _CACHE["last_result"] = res

    full = np.empty((B, S, D), dtype=np.float32)
    for c in range(NCORES):
        b, hg = c // 2, c % 2
        full[b, :, hg * DL:(hg + 1) * DL] = res.results[c]["out"].T
    return full


# revision 61
# speedup vs baseline: 1.0080x; 1.0080x over previous
"""Self-contained Trainium2 Bass kernel for a 12-head attention layer.

Problem: x[4,2048,768] -> attention(QKV projections, softmax, context),
NUM_HEADS=12, SIZE_PER_HEAD=64, additive mask from mask[4,2048].

Sharding over 8 NeuronCores: core c handles batch b=c//2 and head-group
hg=c%2 (6 heads, 384 feature columns).  Everything is local per core:
no collectives.

Design (ACT-bound): the per-core exp work (6 heads x 2048^2 = 25.2M
elements at 1 elem/lane/cycle @1.2GHz + ~350c/inst) floors the Scalar
engine at ~200us busy with N=1024 ACTs, so everything else is arranged
to hide under that roof:

  one flattened software pipeline over all 192 (pair p, f-chunk g of
  512, t-tile ti) steps; per step:
    scores  S^T[t,f]: head A -> psS[:,0:512], head B -> psS[:,512:1024]
            (one [128,1024] psum tile = two banks, double-buffered;
            the two K=64 matmuls row-tile-overlap partially)
    exp     ONE ACT N=1024 over both heads' chunks:
            Exp(psS + adder[ti]) -> bf16 [128,1024]  (mask = ACT bias,
            zero for the all-ones mask)
    ctx     emitted with a LAG=16-step delay so the PE FIFO never
            blocks on the ACT: per head one [65,512] psum accumulated
            over ti; V carries a leading ones-column so psum row 0 is
            the softmax denominator (no separate denominator matmuls);
            segment s's ctx drains exactly during segment s+1's scores,
            keeping the ctx psum tiles single-buffered.  Each segment's
            ti=0 ctx is deferred one extra step (its bank waits on the
            previous epilogue's drain copy); the LAST segment runs at
            lag 3 in the by-then-idle projection psum banks, shrinking
            the post-exp drain from 16 steps to 3
  the Q/K projections for later pairs drip in as one-instruction
  background thunks in the PE slack under the ACT cadence; the first
  four V-projection chains run during the input-DMA window in the
  then-idle scores psum banks (pipelining with the x chunk arrivals),
  the rest as two blocks behind the first segment's scores, hidden by
  the ACT's exp backlog.  Input DMA is ~18us (4.9MB at
  a measured ~276GB/s effective) and is covered by garbage warm-up
  matmuls that release the HAM clock throttle; weights land before x
  so the first projection chains pipeline with the x-chunk arrivals
  (the last chunk's arrival gates the first scores regardless of
  warm-up sizing — measured optimum 30 warm matmuls).

  normalization fully off the PE: DVE psum drain, sbuf-sbuf DMA gather
  of the denominator rows into [128,n] so the reciprocal uses all DVE
  lanes, DMA scatter back to a row, gpsimd partition_broadcast, DVE
  multiply, DMA out.  The last segment uses 256-wide chains so the
  kernel tail pipelines across DVE/DMA/GpSimd.

Output per core: ctx^T [384,2048] f32; host transposes/concats.
Measured: 256.6-257.7us HW exec across 4 healthy-clock runs
(session baseline: 305-361us), rel err 3.468e-3.  NOTE: the chip
throttles ~20% under sustained benchmarking (check ACTIVATE mean
~1045ns healthy vs ~1251ns throttled before comparing numbers).
"""

import numpy as np
import ml_dtypes

B, S, D = 4, 2048, 768
H, DH = 12, 64
HL = 6          # heads per core
DL = HL * DH    # 384 feature columns per core
NCORES = 8
P = 128
KO = 6          # full k-subtiles of the 768 contraction
NT = S // P     # 16 T-tiles
NG = 4          # f-chunks of 512 per head

_CACHE = {}


def _build(with_bias=False, ncores=NCORES, use_adder=True):
    import concourse.mybir as mybir
    import concourse.tile as tile
    from concourse import bacc

    dt = mybir.dt
    Exp = mybir.ActivationFunctionType.Exp
    Alu = mybir.AluOpType

    nc = bacc.Bacc("TRN2", target_bir_lowering=False, debug=False,
                   num_devices=ncores)

    DE = D + 1 if with_bias else D
    WVC = HL * (DH + 1) if with_bias else DL   # 390 vs 384
    # no-bias build ships x token-chunk-major ([p, chunk, ko, t] flat):
    # each 512-token chunk carries ALL 6 k-subtiles, so the first Q/K
    # projection chains complete at chunk-0 arrival instead of waiting
    # for the whole 3.1MB transfer (head -8us)
    xT = nc.dram_tensor("xT", [DE, S] if with_bias else [P, KO * S],
                        dt.bfloat16, kind="ExternalInput")
    wq = nc.dram_tensor("wq", [DE, DL], dt.bfloat16, kind="ExternalInput")
    wk = nc.dram_tensor("wk", [DE, DL], dt.bfloat16, kind="ExternalInput")
    wv = nc.dram_tensor("wv", [DE, WVC], dt.bfloat16, kind="ExternalInput")
    adder = nc.dram_tensor("adder", [P, NT], dt.float32, kind="ExternalInput")
    out = nc.dram_tensor("out", [DL, S], dt.float32, kind="ExternalOutput")

    KE = KO + 1 if with_bias else KO

    with tile.TileContext(nc) as tc:
        with (
            tc.tile_pool(name="persist", bufs=1) as sb,
            tc.tile_pool(name="work", bufs=3) as work,
            tc.tile_pool(name="fin", bufs=2) as fin,
            tc.tile_pool(name="ps_s", bufs=2, space="PSUM") as ps_s,
            tc.tile_pool(name="ps_c", bufs=1, space="PSUM") as ps_c,
        ):
            # ---- input DMA ----
            xTs = sb.tile([P, KE, S], dt.bfloat16, tag="xTs")
            wqs = sb.tile([P, KE, DL], dt.bfloat16, tag="wqs")
            wks = sb.tile([P, KE, DL], dt.bfloat16, tag="wks")
            wvs = sb.tile([P, KE, WVC], dt.bfloat16, tag="wvs")
            adder_sb = sb.tile([P, NT], dt.float32, tag="adder")

            def dma_w(w_dram, w_sb, cols):
                # host already stores rows as [p, ko, m] (partition-major),
                # so this is a contiguous transfer, not a gather
                nc.sync.dma_start(
                    w_sb[:, 0:KO, 0:cols],
                    w_dram.ap()[0:D, :].rearrange("(p ko) m -> p ko m", p=P))
                if with_bias:
                    nc.sync.dma_start(w_sb[0:1, KO, 0:cols],
                                      w_dram.ap()[D:D + 1, :])

            # weights first: the first projection chains then pipeline
            # with the xT chunk arrivals instead of waiting for all of x
            dma_w(wq, wqs, DL)
            dma_w(wk, wks, DL)
            if with_bias:
                dma_w(wv, wvs, WVC)
                nc.sync.dma_start(adder_sb[:], adder.ap())
                for ko in range(KO):
                    nc.sync.dma_start(
                        xTs[:, ko, :], xT.ap()[ko * P:(ko + 1) * P, :])
                nc.sync.dma_start(xTs[0:1, KO, :], xT.ap()[D:D + 1, :])
            else:
                CW = KO * 512
                nc.sync.dma_start(
                    xTs[:, 0:KO, 0:512],
                    xT.ap()[:, 0:CW].rearrange("p (ko t) -> p ko t", ko=KO))
                dma_w(wv, wvs, WVC)
                nc.sync.dma_start(adder_sb[:], adder.ap())
                for c in range(1, 4):
                    nc.sync.dma_start(
                        xTs[:, 0:KO, c * 512:(c + 1) * 512],
                        xT.ap()[:, c * CW:(c + 1) * CW].rearrange(
                            "p (ko t) -> p ko t", ko=KO))

            # persistent projection outputs
            qt = sb.tile([P, 3, S], dt.bfloat16, tag="qt")   # Q^T/8 (+bias)
            kt = sb.tile([P, 3, S], dt.bfloat16, tag="kt")   # K^T (+bias)
            # V' token-major, 65-col head blocks (65th col = ones -> denom)
            # ones column FIRST in each head block: the ctx psum row 0 is
            # then the softmax denominator at partition 0, so the epilogue
            # reciprocal/broadcast needs no partition-moving DMAs
            vp = sb.tile([P, NT, HL, DH + 1], dt.bfloat16, tag="vp")
            if not with_bias:
                nc.gpsimd.memset(vp[:, :, :, 0:1], 1.0)

            # ---- projections ----
            def proj_qk(w_sb, dst, m, ns=(0, 1, 2, 3), tag="proj"):
                for n in ns:
                    pt = ps_c.tile([P, 512], dt.float32, tag=tag,
                                   name="pt", bufs=2 if tag == "proj" else 1)
                    for k in range(KE):
                        lhsT = (w_sb[:, k, m * P:(m + 1) * P] if k < KO
                                else w_sb[0:1, k, m * P:(m + 1) * P])
                        rhs = (xTs[:, k, n * 512:(n + 1) * 512] if k < KO
                               else xTs[0:1, k, n * 512:(n + 1) * 512])
                        nc.tensor.matmul(pt[:], lhsT, rhs,
                                         start=(k == 0), stop=(k == KE - 1))
                    nc.vector.tensor_copy(dst[:, m, n * 512:(n + 1) * 512],
                                          pt[:])

            def proj_v_into(mt, pt, off):
                for k in range(KE):
                    lhsT = (xTs[:, k, mt * P:(mt + 1) * P] if k < KO
                            else xTs[0:1, k, mt * P:(mt + 1) * P])
                    rhs = wvs[:, k, 0:WVC] if k < KO else wvs[0:1, k, 0:WVC]
                    nc.tensor.matmul(pt[:, off:off + WVC], lhsT, rhs,
                                     start=(k == 0), stop=(k == KE - 1))
                if with_bias:
                    nc.vector.tensor_copy(
                        vp[:, mt, :, :],
                        pt[:, off:off + WVC].rearrange("p (h c) -> p h c",
                                                       h=HL))
                else:
                    nc.vector.tensor_copy(
                        vp[:, mt, :, 1:DH + 1],
                        pt[:, off:off + DL].rearrange("p (h c) -> p h c",
                                                      h=HL))

            def proj_v(mt):
                pt = ps_c.tile([P, 512], dt.float32, tag="proj", name="pt",
                               bufs=2)
                proj_v_into(mt, pt, 0)

            def epilogue(p, g, ctx_ps, fine=False):
                # ctx_ps: [hip] -> psum tile; row 0 = denominator,
                # rows 1..64 = ctx.  Drain psum fast (DVE), then gather the
                # denom rows to [128,n] via sbuf-sbuf DMA so the reciprocal
                # uses all DVE lanes, scatter back, gpsimd-broadcast,
                # multiply, DMA out.  fine=True splits into 256-wide chains
                # so the kernel tail pipelines.
                csts = []
                for hip in range(2):
                    cst = fin.tile([DH + 1, 512], dt.float32,
                                   tag=f"cst{hip}", name="cst", bufs=2)
                    nc.vector.tensor_copy(cst[:], ctx_ps[hip][0:DH + 1, :])
                    csts.append(cst)
                nch = 2 if fine else 1
                cw = 512 // nch
                for ch in range(nch):
                    dcol = fin.tile([P, 8 // nch], dt.float32,
                                    tag=f"dcol{nch}", name="dcol",
                                    bufs=2 * nch)
                    for hip in range(2):
                        nc.sync.dma_start(
                            dcol[:, hip * 4 // nch:(hip + 1) * 4 // nch],
                            csts[hip][0:1, ch * cw:(ch + 1) * cw])
                    rc = fin.tile([P, 8 // nch], dt.float32,
                                  tag=f"rc{nch}", name="rc", bufs=2 * nch)
                    nc.vector.reciprocal(rc[:], dcol[:])
                    for hip in range(2):
                        rrow = fin.tile([1, cw], dt.float32,
                                        tag=f"rrow{nch}", name="rrow",
                                        bufs=2 * nch)
                        nc.sync.dma_start(
                            rrow[:],
                            rc[:, hip * 4 // nch:(hip + 1) * 4 // nch])
                        bb = fin.tile([DH + 1, cw], dt.float32,
                                      tag=f"bb{nch}", name="bb",
                                      bufs=2 * nch)
                        nc.gpsimd.partition_broadcast(bb[:], rrow[:])
                        otc = fin.tile([DH + 1, cw], dt.float32,
                                       tag=f"otc{nch}", bufs=3 * nch,
                                       name="otc")
                        nc.vector.tensor_tensor(
                            otc[:, :],
                            csts[hip][:, ch * cw:(ch + 1) * cw],
                            bb[:, :], Alu.mult)
                        nc.sync.dma_start(
                            out.ap()[p * P + hip * DH:
                                     p * P + (hip + 1) * DH,
                                     g * 512 + ch * cw:
                                     g * 512 + (ch + 1) * cw],
                            otc[1:DH + 1, :])

            def attn_all(bg, pre_step):
                # One flattened software pipeline over all 192
                # (pair, g, ti) steps: scores+exp for step j, ctx for step
                # j-16.  The segment-sized lag means ctx deps are a full
                # exp-backlog old (the PE FIFO never waits on the ACT),
                # segment s's ctx drains exactly during segment s+1's
                # scores (ctx psum tiles stay single-buffered), and every
                # segment/pair boundary bubble disappears.  pre_step maps
                # step -> emission block run before that step (V-proj
                # blocks that hide under the exp backlog); bg interleaves
                # one background-projection instruction per step.
                LAG = 16
                LATE = 176   # last segment: lag 3, ctx lives in the proj
                             # banks (free after the last projection), so
                             # the post-exp ctx drain shrinks 16 -> 3 steps
                ctx_tiles = {}
                exp_tiles = {}

                def seg_of(step):
                    p, r = divmod(step, 64)
                    return p, r // 16, r % 16

                def emit_at(j):
                    if j >= LATE:
                        return j + 3
                    # defer ti==0 one step: its psum tile waits on the
                    # previous segment's epilogue drain copy (bank WAR);
                    # emitting it a step later keeps the PE FIFO clear
                    return j + LAG + (1 if j % 16 == 0 else 0)

                for step in range(192 + LAG):
                    if step in pre_step:
                        pre_step[step]()
                    if step < 192:
                        p, g, ti = seg_of(step)
                        psS = ps_s.tile([P, 1024], dt.float32, tag="s",
                                        name="psS")
                        nc.tensor.matmul(
                            psS[:, 0:512],
                            kt[0:DH, p, ti * P:(ti + 1) * P],
                            qt[0:DH, p, g * 512:(g + 1) * 512],
                            start=True, stop=True)
                        nc.tensor.matmul(
                            psS[:, 512:1024],
                            kt[DH:P, p, ti * P:(ti + 1) * P],
                            qt[DH:P, p, g * 512:(g + 1) * 512],
                            start=True, stop=True)
                        et = work.tile([P, 1024], dt.bfloat16, tag="exp",
                                       name="et", bufs=LAG + 2)
                        nc.scalar.activation(
                            et[:], psS[:], Exp,
                            bias=(adder_sb[:, ti:ti + 1] if use_adder
                                  else 0.0),
                            scale=1.0)
                        exp_tiles[step] = et
                    for j in range(max(0, step - LAG - 1), step + 1):
                        if emit_at(j) != step or j >= 192:
                            continue
                        p, g, ti = seg_of(j)
                        seg = 4 * p + g
                        if ti == 0:
                            if j >= LATE:
                                ctx_tiles[seg] = [
                                    ps_c.tile([P, 512], dt.float32,
                                              tag="proj", name="ctx_ps",
                                              bufs=2)
                                    for hip in range(2)
                                ]
                            else:
                                ctx_tiles[seg] = [
                                    ps_c.tile([P, 512], dt.float32,
                                              tag=f"c{hip}",
                                              name="ctx_ps")
                                    for hip in range(2)
                                ]
                        et = exp_tiles.pop(j)
                        for hip in range(2):
                            nc.tensor.matmul(
                                ctx_tiles[seg][hip][0:DH + 1, :],
                                vp[:, ti, 2 * p + hip, :],
                                et[:, hip * 512:(hip + 1) * 512],
                                start=(ti == 0), stop=(ti == NT - 1))
                        if ti == NT - 1:
                            epilogue(p, g, ctx_tiles.pop(seg),
                                     fine=(seg == 11))
                    pops = 2 if step < 40 else 1
                    for _ in range(pops):
                        if bg:
                            bg.pop(0)()

            # PE warm-up: garbage matmuls with no input deps run during the
            # initial DMA wait, releasing the HAM clock throttle.
            warm = sb.tile([P, 512], dt.bfloat16, tag="warm")
            nc.gpsimd.memset(warm[:], 0.0)
            wexp = sb.tile([P, 1], dt.bfloat16, tag="wexp")
            nc.scalar.activation(wexp[:], warm[:, 0:1], Exp)
            wpt = ps_s.tile([P, 1024], dt.float32, tag="s", name="wpt")
            for wi in range(30):
                nc.tensor.matmul(wpt[:, 0:512], warm[:, 0:P], warm[:],
                                 start=(wi == 0), stop=(wi == 29))

            def proj_thunks_qk(w_sb, dst, m, n):
                # one-instruction-per-thunk version of proj_qk(m, (n,))
                state = {}

                def mk(k):
                    def t():
                        if k == 0:
                            state["pt"] = ps_c.tile([P, 512], dt.float32,
                                                    tag="proj", name="pt",
                                                    bufs=2)
                        lhsT = (w_sb[:, k, m * P:(m + 1) * P] if k < KO
                                else w_sb[0:1, k, m * P:(m + 1) * P])
                        rhs = (xTs[:, k, n * 512:(n + 1) * 512] if k < KO
                               else xTs[0:1, k, n * 512:(n + 1) * 512])
                        nc.tensor.matmul(state["pt"][:], lhsT, rhs,
                                         start=(k == 0), stop=(k == KE - 1))
                    return t

                def cp():
                    nc.vector.tensor_copy(
                        dst[:, m, n * 512:(n + 1) * 512], state["pt"][:])

                return [mk(k) for k in range(KE)] + [cp]

            # prefix: only what scores(0..3) need up front; remaining k-m0
            # chunks drip in just ahead of their ti via the bg queue (2
            # pops/step early on); the V projections run AFTER g0's first
            # scores, hidden under the ACT's exp backlog.
            proj_qk(wqs, qt, 0, ns=(0,))
            proj_qk(wks, kt, 0, ns=(0,), tag="c0")
            proj_qk(wks, kt, 0, ns=(1,), tag="c1")
            proj_qk(wks, kt, 0, ns=(2,))
            for vb in range(2):
                wv_ps = ps_s.tile([P, 1024], dt.float32, tag="s",
                                  name="wvps")
                for half in range(2):
                    proj_v_into(2 * vb + half, wv_ps, half * 512)
            proj_qk(wks, kt, 0, ns=(3,))

            def v_block(lo, hi):
                def f():
                    for mt in range(lo, hi):
                        proj_v(mt)
                return f

            def v_one(mt, tag):
                # single V chain in a briefly-idle ctx bank (free between
                # the head k-projection chains and ctx(0) at step ~17)
                def f():
                    pt = ps_c.tile([P, 512], dt.float32, tag=tag,
                                   name="pt", bufs=1)
                    proj_v_into(mt, pt, 0)
                return f

            bg = []
            for n in (1, 2, 3):
                bg += proj_thunks_qk(wqs, qt, 0, n)
            for n in range(4):
                bg += proj_thunks_qk(wks, kt, 1, n)
            for n in range(4):
                bg += proj_thunks_qk(wqs, qt, 1, n)
            for n in range(4):
                bg += proj_thunks_qk(wks, kt, 2, n)
            for n in range(4):
                bg += proj_thunks_qk(wqs, qt, 2, n)
            bg += [lambda: None] * (300 - len(bg))
            attn_all(bg, {16: v_block(4, 10), 24: v_block(10, NT)})

    nc.compile()
    return nc


def _prep_core_inputs(c, x, Wq, bq, Wk, bk, Wv, bv, mask, with_bias):
    bf16 = ml_dtypes.bfloat16
    b, hg = c // 2, c % 2
    cols = slice(hg * DL, (hg + 1) * DL)
    DE = D + 1 if with_bias else D

    if with_bias:
        xT_aug = np.empty((DE, S), dtype=bf16)
        xT_aug[:D] = x[b].T.astype(bf16)
        xT_aug[D] = np.float32(1.0)
    else:
        # [p, chunk, ko, t] flat: each 512-token chunk carries all six
        # 128-row k-subtiles (content identical to the on-chip xTs view)
        xT_aug = np.ascontiguousarray(
            x[b].T.astype(bf16).reshape(KO, P, 4, 512)
            .transpose(1, 2, 0, 3).reshape(P, KO * S))

    def pko(w):
        # store weight rows partition-major ([p, ko] instead of [ko, p])
        # so the on-chip DMA into [P, KO, cols] is contiguous
        c = w.shape[1]
        return w.reshape(KO, P, c).transpose(1, 0, 2).reshape(D, c)

    wq_aug = np.empty((DE, DL), dtype=bf16)
    wq_aug[:D] = pko((Wq[:, cols] / 8.0).astype(bf16))
    wk_aug = np.empty((DE, DL), dtype=bf16)
    wk_aug[:D] = pko(Wk[:, cols].astype(bf16))
    if with_bias:
        wq_aug[D] = (bq[cols] / 8.0).astype(bf16)
        wk_aug[D] = bk[cols].astype(bf16)
        wv_aug = np.zeros((DE, HL * (DH + 1)), dtype=bf16)
        wv_loc = Wv[:, cols].astype(np.float32)
        bv_loc = bv[cols].astype(np.float32)
        for j in range(HL):
            wv_aug[:D, j * (DH + 1) + 1:(j + 1) * (DH + 1)] = \
                wv_loc[:, j * DH:(j + 1) * DH].astype(bf16)
            wv_aug[D, j * (DH + 1) + 1:(j + 1) * (DH + 1)] = \
                bv_loc[j * DH:(j + 1) * DH].astype(bf16)
            wv_aug[D, j * (DH + 1)] = np.float32(1.0)
        wv_aug[:D] = pko(wv_aug[:D].copy())
    else:
        wv_aug = np.empty((DE, DL), dtype=bf16)
        wv_aug[:D] = pko(Wv[:, cols].astype(bf16))

    add = ((mask[b].astype(np.float32) - 1.0) * 10000.0)
    adder_t = add.reshape(NT, P).T.copy()   # [128,16]: [p, ti]

    return {"xT": xT_aug, "wq": wq_aug, "wk": wk_aug, "wv": wv_aug,
            "adder": np.ascontiguousarray(adder_t, dtype=np.float32)}


def kernel(x, Wq, bq, Wk, bk, Wv, bv, mask, _trace=False):
    from concourse.bass_utils import run_bass_kernel_spmd

    x = np.asarray(x, dtype=np.float32)
    Wq = np.asarray(Wq, dtype=np.float32)
    bq = np.asarray(bq, dtype=np.float32)
    Wk = np.asarray(Wk, dtype=np.float32)
    bk = np.asarray(bk, dtype=np.float32)
    Wv = np.asarray(Wv, dtype=np.float32)
    bv = np.asarray(bv, dtype=np.float32)
    mask = np.asarray(mask)

    with_bias = bool(bq.any() or bk.any() or bv.any())
    use_adder = not bool(np.asarray(mask).all())
    key = ("nc", with_bias, use_adder)
    if key not in _CACHE:
        _CACHE[key] = _build(with_bias=with_bias, use_adder=use_adder)
    nc = _CACHE[key]

    in_maps = [_prep_core_inputs(c, x, Wq, bq, Wk, bk, Wv, bv, mask,
                                 with_bias)
               for c in range(NCORES)]
    res = run_bass_kernel_spmd(nc, in_maps, core_ids=list(range(NCORES)),
                               trace=_trace)
    if _trace:
        _CACHE["last_result"] = res

    full = np.empty((B, S, D), dtype=np.float32)
    for c in range(NCORES):
        b, hg = c // 2, c % 2
        full[b, :, hg * DL:(hg + 1) * DL] = res.results[c]["out"].T
    return full
